# revision 1
# baseline (speedup 1.0000x reference)
"""Trainium2 Bass kernel for the DLI (dialogue-turn ordering) loss.

Math (exact reduction of the reference):
  With 2 classes, NLL(label y) = softplus(l_{1-y} - l_y).
  u[b,j] = enc[b,j] @ (W[:D,1]-W[:D,0]),
  v[b,k] = enc[b,k] @ (W[D:,1]-W[D:,0]),
  c      = b[1]-b[0],  d[b,j,k] = u[b,j] + v[b,k] + c
  label = 1 iff k == j-1; valid pairs: k < j < len_b;  softplus(-d) = softplus(d) - d
  =>  sum_nll = sum_{valid} softplus(d) - sum_{b, 1<=j<len_b} d[b,j,j-1]
  loss = sum_nll / max(n_valid, 1)

Sharding: data-parallel over batch (64 -> 8 cores x 8). Each core emits one
f32 partial sum; the host adds the 8 partials and divides by the exact
n_valid from the mask. The tiny derived tensors (wu/wv rows in bf16, the
additive row masks rmM = [j<len]?c:NEG+c and diagonal validity masks rm1/rm2)
are precomputed on the host and shipped as small extra inputs — the 64MB
encoder tensor is the memory-bound payload and stays on-device.

Engine split per core (target_regime=memory; HBM floor ~23us for 8MB):
  DMA:    enc arrives bf16 via SWDGE casting DMAs (HBM reads stay f32/8MB);
          w rows broadcast down partitions by a stride-0-source DMA
  DVE:    6 u-dots as fused STT(+accum); bf16 2x-mode multiplies for the
          other 10 dots; small phase-B masking ops
  ACT:    10 dot reductions (Copy+accum_out), per-pair Exp (bias folds
          u+rowmask+c, reads PSUM) and Ln(x+1) with fused row-sum
  PE:     v-column transposes + broadcast matmuls building d[j,k] in PSUM
  GpSimd: SWDGE DMA triggers + one-time memset/affine consts only (no ucode
          ops: they force Q7 dge-drains against in-flight SWDGE DMAs, and big
          GpSimd tensor ops crawl while DVE streams -- shared SBUF ports)
All ACT functions (Copy/Exp/Ln) are forced into the single
natural_log_exp_and_others table so the act-table is loaded exactly once.
"""

import glob
import json
import os
import shutil
import sys
import tempfile

if "/opt/trn_rl_repo" not in sys.path:
    sys.path.insert(0, "/opt/trn_rl_repo")


def _force_combined_act_table():
    """Point walrus at an act_info.json holding only natural_log_exp_and_others
    (contains exp+ln+copy), so every ACTIVATE shares one table."""
    if os.environ.get("BASS_ACT_ROOT_JSON_PATH"):
        return
    from neuronxcc.driver.Job import Job  # type: ignore

    pwp = None
    for cand in glob.glob(os.path.join(Job.getPackageDir(), "pwp", "pwp_bin_*")):
        if os.path.exists(os.path.join(cand, "act_info.json")):
            pwp = cand
            break
    if pwp is None:
        return
    info = json.load(open(os.path.join(pwp, "act_info.json")))
    keep = [t for t in info.get("act_func_sets", [])
            if t.get("name") == "natural_log_exp_and_others"]
    if not keep:
        return
    out_dir = os.path.join(tempfile.gettempdir(), "dli_act_combined")
    os.makedirs(out_dir, exist_ok=True)
    for t in keep:
        for k in info.get("pwp_file_keys", []):
            f = t.get(k)
            src = os.path.join(pwp, f) if f else None
            if src and os.path.exists(src):
                dst = os.path.join(out_dir, f)
                if not os.path.exists(dst):
                    shutil.copy(src, dst)
    info = dict(info)
    info["act_func_sets"] = keep
    with open(os.path.join(out_dir, "act_info.json"), "w") as f:
        json.dump(info, f)
    os.environ["BASS_ACT_ROOT_JSON_PATH"] = os.path.join(out_dir, "act_info.json")


_force_combined_act_table()

from contextlib import ExitStack

import ml_dtypes
import numpy as np

import concourse.bacc as bacc
import concourse.bass as bass
import concourse.hw_specs as hw_specs
import concourse.mybir as mybir
import concourse.tile as tile
from concourse.masks import make_identity

# Make bass's act-table placement agree with the trimmed act_info.json walrus
# sees: only the combined exp+ln+copy table exists, so every ACTIVATE maps to
# act_func_set_id 0 and the table is loaded exactly once.
_orig_get_act_tables = hw_specs.get_activation_tables


def _combined_act_tables(module_arch):
    tabs = _orig_get_act_tables(module_arch)
    kept = {k: v for k, v in tabs.items() if k == "natural_log_exp_and_others"}
    return kept if kept and os.environ.get("BASS_ACT_ROOT_JSON_PATH") else tabs


hw_specs.get_activation_tables = _combined_act_tables
bacc.get_activation_tables = _combined_act_tables

# Cheaper kernel teardown: drain + one all-engine barrier + sem clear. The
# stock epilogue adds a second all-engine barrier after the clear; engines
# that pass the first barrier only run their terminal branch, and the next
# execution starts only after every engine (incl. the clearing one) halts,
# so the second barrier only adds ~4us of EVSEM latency.
from concourse.vector_clock import ScopedClock as _ScopedClock


def _cheap_drain_and_barrier(self, tick_clock, wait_clock):
    drain_inst = self.nc.sync.drain()
    wait_clock.add_sem_waits(
        drain_inst.ins, _ScopedClock({None: tick_clock.global_clock})
    )
    self.nc.all_engine_barrier()
    popped = self.nc._tile_sem_poison_stack.pop()
    assert popped is self._sem_poison
    self.nc.clear_and_free_semaphores(list(self.sems.allocated().values()))


tile.TileContext._drain_and_barrier = _cheap_drain_and_barrier

F32 = mybir.dt.float32
BF16 = mybir.dt.bfloat16
ALU = mybir.AluOpType
ACTF = mybir.ActivationFunctionType
AXX = mybir.AxisListType.X

BSZ, L, D = 64, 128, 2048
N_CORES = 8
NB = BSZ // N_CORES  # batches per core
NEG = -30000.0  # additive row-mask value; exp(NEG) == 0 in f32
STT_SET = {0, 1, 2}  # u-dots as fused DVE STT (rest: DVE mul + ACT accum)
V_STT_SET = {6, 7}  # v-dots as DVE STT so the tail isn't gated on ACT reduces
CHUNKS = [1, 1, 2, 2, 1, 1]  # batches per DMA chunk (small at both ends)
LN_GROUPS = [[0, 1, 2, 3], [4, 5, 6], [7]]  # batched Ln reductions (last alone: short tail)


def build_program():
    nc = bacc.Bacc("TRN2", target_bir_lowering=False, debug=False, num_devices=1)

    enc = nc.dram_tensor("enc", [NB, L, D], F32, kind="ExternalInput").ap()
    wuv = nc.dram_tensor("wuv", [2, D], BF16, kind="ExternalInput").ap()
    aux = nc.dram_tensor("aux", [L, 3 * NB], F32, kind="ExternalInput").ap()
    out = nc.dram_tensor("out", [1, 1], F32, kind="ExternalOutput").ap()

    with tile.TileContext(nc) as tc, ExitStack() as ctx:
        consts = ctx.enter_context(tc.tile_pool(name="consts", bufs=1))
        accs = ctx.enter_context(tc.tile_pool(name="accs", bufs=1))
        enc_pool = ctx.enter_context(tc.tile_pool(name="enc", bufs=3))
        junk_pool = ctx.enter_context(tc.tile_pool(name="junk", bufs=3))
        prod_pool = ctx.enter_context(tc.tile_pool(name="prod", bufs=4))
        d2_pool = ctx.enter_context(tc.tile_pool(name="d2", bufs=3))
        rows_pool = ctx.enter_context(tc.tile_pool(name="rows", bufs=3))
        psum_d_pool = ctx.enter_context(tc.tile_pool(name="psd", bufs=3, space="PSUM"))
        psum_v_pool = ctx.enter_context(tc.tile_pool(name="psv", bufs=2, space="PSUM"))
        psum_misc = ctx.enter_context(tc.tile_pool(name="psm", bufs=1, space="PSUM"))

        # ---- w rows broadcast down all partitions via stride-0-source DMAs ----
        wv_b = consts.tile([L, D], BF16)
        nc.sync.dma_start(wv_b[:], wuv[1:2, :].broadcast_to([L, D]))
        wu_b = consts.tile([L, D], BF16)
        nc.sync.dma_start(wu_b[:], wuv[0:1, :].broadcast_to([L, D]))
        aux_sb = consts.tile([L, 3 * NB], F32)
        nc.sync.dma_start(aux_sb[:], aux[:])
        rmM_all = aux_sb[:, 0:NB]          # (j<len_b ? 0 : NEG) + c
        rm1_all = aux_sb[:, NB : 2 * NB]   # [1<=j<len_b]
        rm2_all = aux_sb[:, 2 * NB : 3 * NB]  # [j<len_b-1]

        # ---- enc loads: SWDGE casting DMAs, CHUNKS batches per chunk ----
        enc_tiles = {}
        chunk_of = {}
        b0 = 0
        for ci, tb in enumerate(CHUNKS):
            chunk = enc_pool.tile([L, tb * D], BF16, tag=f"enc{ci % 3}", name=f"encc{ci}")
            if tb == 1:
                nc.gpsimd.dma_start(chunk[:], enc[b0])
            else:
                nc.gpsimd.dma_start(
                    chunk[:].rearrange("l (b d) -> l b d", b=tb),
                    enc[b0 : b0 + tb].rearrange("b l d -> l b d"),
                )
            for i in range(tb):
                enc_tiles[b0 + i] = chunk[:, i * D : (i + 1) * D]
                chunk_of[b0 + i] = ci
            b0 += tb

        # ---- constants ----
        ones_row = consts.tile([1, L], F32)
        nc.gpsimd.memset(ones_row[:], 1.0)
        ones_col = consts.tile([L, 1], F32)
        nc.gpsimd.memset(ones_col[:], 1.0)
        ident = consts.tile([L, L], F32)
        make_identity(nc, ident[:])
        # multiplicative lower-triangular mask: tri01[j,k] = 1 if k<j else 0
        tri01 = consts.tile([L, L], F32)
        nc.gpsimd.memset(tri01[:], 1.0)
        nc.gpsimd.affine_select(
            out=tri01[:], in_=tri01[:], compare_op=ALU.is_gt, fill=0.0,
            base=0, pattern=[[-1, L]], channel_multiplier=1,
        )

        # ---- main pipeline ----
        UV = accs.tile([L, 2 * NB], F32)  # cols 0..NB-1 = u_b; NB..2NB-1 = v_b
        n_groups = len(LN_GROUPS)
        grp_of = {b: (g, q) for g, grp in enumerate(LN_GROUPS) for q, b in enumerate(grp)}
        RS = accs.tile([L, n_groups], F32)
        exg_pool = ctx.enter_context(tc.tile_pool(name="exg", bufs=1))
        exg_tiles = [exg_pool.tile([L, len(grp) * L], F32, tag=f"exg{i}", name=f"exg{i}")
                     for i, grp in enumerate(LN_GROUPS)]

        def dot_stt(enc_ap, w_tile, acc_col):
            junk = junk_pool.tile([L, D], BF16)
            nc.vector.scalar_tensor_tensor(
                out=junk[:], in0=enc_ap, scalar=1.0, op0=ALU.mult,
                in1=w_tile[:], op1=ALU.mult, accum_out=acc_col,
            )

        def dot_act(enc_ap, w_tile, acc_col):
            prod = prod_pool.tile([L, D], BF16)
            nc.vector.tensor_mul(prod[:], enc_ap, w_tile[:])
            junk = junk_pool.tile([L, D], BF16, tag="junk_act")
            nc.scalar.activation(junk[:], prod[:], ACTF.Copy, accum_out=acc_col)

        def phase_b(b):
            v_col = UV[:, NB + b : NB + b + 1]
            psum_v = psum_v_pool.tile([1, L], F32)
            nc.tensor.matmul(psum_v[:], lhsT=v_col, rhs=ident[:], is_transpose=True)
            v_row = rows_pool.tile([1, L], F32, tag="vrow")
            nc.vector.tensor_copy(v_row[:], psum_v[:])
            psum_d = psum_d_pool.tile([L, L], F32)
            nc.tensor.matmul(psum_d[:], lhsT=ones_row[:], rhs=v_row[:])
            su = rows_pool.tile([L, 1], F32, tag="su")
            nc.vector.tensor_add(su[:], UV[:, b : b + 1], rmM_all[:, b : b + 1])
            ex = d2_pool.tile([L, L], F32, tag="ex")
            nc.scalar.activation(ex[:], psum_d[:], ACTF.Exp, bias=su[:, 0:1])
            g, q = grp_of[b]
            nc.vector.tensor_mul(exg_tiles[g][:, q * L : (q + 1) * L], ex[:], tri01[:])

        def ln_group(g):
            sp = d2_pool.tile([L, len(LN_GROUPS[g]) * L], F32, tag="sp")
            nc.scalar.activation(sp[:], exg_tiles[g][:], ACTF.Ln, bias=1.0,
                                 accum_out=RS[:, g : g + 1])

        done_groups = set()
        b0 = 0
        for tb in CHUNKS:
            batches = range(b0, b0 + tb)
            for b in batches:
                # v-dots first: they gate phase B. Tail batches use DVE STT
                # so phase B isn't queued behind ACT reductions.
                if b in V_STT_SET:
                    dot_stt(enc_tiles[b], wv_b, UV[:, NB + b : NB + b + 1])
                else:
                    dot_act(enc_tiles[b], wv_b, UV[:, NB + b : NB + b + 1])
            for b in batches:
                if b in STT_SET:
                    dot_stt(enc_tiles[b], wu_b, UV[:, b : b + 1])
                else:
                    dot_act(enc_tiles[b], wu_b, UV[:, b : b + 1])
            for b in batches:
                phase_b(b)
            b0 += tb
            for g, grp in enumerate(LN_GROUPS):
                if g not in done_groups and grp[-1] < b0:
                    ln_group(g)
                    done_groups.add(g)

        # ---- diagonal (label-1) terms, all batches at once ----
        # diag sum = sum_j u[j]*rm1[j] + sum_k (v[k]+c)*rm2[k]; the c*rm2 part
        # equals c*(len-1) and is folded in on the host via rmM's c... no:
        # rm2 carries plain 0/1; vc adds nothing here because c is folded into
        # rmM (bias path). The diagonal needs v+c explicitly, so the host puts
        # c into rm2's companion: we compute sum v*rm2 and the host adds
        # c*(len_b-1) terms into its final combine.
        dUV = accs.tile([L, 2 * NB], F32)
        nc.vector.tensor_mul(dUV[:], UV[:, 0 : 2 * NB], aux_sb[:, NB : 3 * NB])

        # ---- final reduction ----
        accA = accs.tile([L, 1], F32)
        nc.vector.reduce_sum(accA[:], RS[:], axis=AXX)
        dr = accs.tile([L, 1], F32)
        nc.vector.reduce_sum(dr[:], dUV[:], axis=AXX)
        nc.vector.tensor_sub(accA[:], accA[:], dr[:])
        psum_s = psum_misc.tile([1, 1], F32, tag="psm")
        nc.tensor.matmul(psum_s[:], lhsT=accA[:], rhs=ones_col[:])
        out_t = accs.tile([1, 1], F32)
        nc.vector.tensor_copy(out_t[:], psum_s[:])
        nc.sync.dma_start(out[:], out_t[:])

    nc.compile()
    return nc


_NC = None


def _get_nc():
    global _NC
    if _NC is None:
        _NC = build_program()
    return _NC


def _prep(encoder_output, mask, W, b):
    """Host-side prep: shard + derived small tensors."""
    W = np.asarray(W, dtype=np.float32)
    b = np.asarray(b, dtype=np.float32).reshape(2)
    mask = np.asarray(mask)
    c = float(b[1] - b[0])
    wuv = np.stack([W[:D, 1] - W[:D, 0], W[D:, 1] - W[D:, 0]]).astype(ml_dtypes.bfloat16)
    lens = mask.astype(np.int64).sum(axis=1)  # [BSZ]
    j = np.arange(L)
    maps = []
    diag_c = 0.0  # host part of the diagonal c-terms: sum_b c*(len_b-1)
    for cid in range(N_CORES):
        sl = slice(cid * NB, (cid + 1) * NB)
        lc = lens[sl]  # [NB]
        rmM = np.where(j[:, None] < lc[None, :], 0.0, NEG).astype(np.float32) + c
        rm1 = ((j[:, None] >= 1) & (j[:, None] < lc[None, :])).astype(np.float32)
        rm2 = (j[:, None] < (lc[None, :] - 1)).astype(np.float32)
        aux = np.concatenate([rmM, rm1, rm2], axis=1)  # [L, 3*NB]
        maps.append(
            {
                "enc": np.ascontiguousarray(encoder_output[sl], dtype=np.float32),
                "wuv": wuv,
                "aux": np.ascontiguousarray(aux),
            }
        )
    diag_c = float(c * (lens - 1).sum())
    n_valid = int((lens * (lens - 1) // 2).sum())
    return maps, diag_c, n_valid


def kernel(encoder_output, mask, W, b, _run_kwargs=None):
    from concourse.bass_utils import run_bass_kernel_spmd

    nc = _get_nc()
    maps, diag_c, n_valid = _prep(np.asarray(encoder_output), mask, W, b)
    res = run_bass_kernel_spmd(nc, maps, core_ids=list(range(N_CORES)),
                               **(_run_kwargs or {}))
    total = float(sum(np.float64(r["out"][0, 0]) for r in res.results))
    total -= diag_c
    loss = total / max(n_valid, 1)
    out = np.array(loss, dtype=np.float32)
    if _run_kwargs is not None:
        return out, res
    return out



# revision 12
# speedup vs baseline: 1.0959x; 1.0959x over previous
"""Trainium2 Bass kernel for the DLI (dialogue-turn ordering) loss — v2.

Math (exact reduction of the reference):
  With 2 classes, NLL(label y) = softplus(l_{1-y} - l_y).
  u[b,j] = enc[b,j] @ (W[:D,1]-W[:D,0]),
  v[b,k] = enc[b,k] @ (W[D:,1]-W[D:,0]),
  c      = b[1]-b[0],  d[b,j,k] = u[b,j] + v[b,k] + c
  label = 1 iff k == j-1; valid pairs: k < j < len_b;  softplus(-d) = softplus(d) - d
  =>  sum_nll = sum_{valid} softplus(d) - sum_{b, 1<=j<len_b} d[b,j,j-1]
  loss = sum_nll / max(n_valid, 1)

v2 layout (ragged packing + PE dots):
  Only rows j < len_b ever matter (arch is ragged_sequence).  The host
  length-balances the 64 batches into 8 cores of 8, packs each core's valid
  rows contiguously, and ships them TRANSPOSED: encT [D=2048, R] f32 where
  R = packed rows padded to a multiple of 128 (512 for the expected data).
  HBM traffic is halved vs the full [8, 128, 2048] and the transposed layout
  lets the TensorEngine do the dots:

    psum_uv[2, R] += wuvT_chunk[128, 2].T @ encT_chunk[128, R]   (16 chunks)

  as f32r matmuls (1 cycle/row at R >= 256), replacing the DVE/ACT dot
  pipeline of v1 that was the bottleneck (ACT 30us + DVE 30us busy).  All
  DMAs are plain-dtype HWDGE on the sync queue (no SWDGE => no GpSimd
  descriptor work, no dge drains).

  Phase B on packed coordinates: tiles of 128 rows; pair blocks (a,a) and
  (a,a-1) only (a batch spans at most 2 tiles).  d is built additively in
  PSUM by rank-1 fp16 matmuls:
    diag:  ident@triNEG + ones@v16 + ones@bp16      (tri kills k>=j)
    off :  ones@v16 + ones@bp16                     (all global k < j)
  where bp16[k] = +A*bid[k] (A=512, fp16-exact) or NEG for padding, and the
  Exp bias column carries u[j] + c - A*bid[j] in f32 (via a K=1 PE
  transpose of the u row).  Cross-batch pairs get exp(d - A*dbid) -> 0;
  same-batch pairs cancel the A terms exactly.  Exp/Ln(1+x) on ACT with
  fused row-sum accumulation; label-1 diagonal handled as masked row sums
  of raw u,v (masks shipped negated so the final PSUM matmul accumulates
  the subtraction).  Host divides by exact n_valid and fixes the c terms.
"""

import glob
import json
import os
import shutil
import sys
import tempfile

if "/opt/trn_rl_repo" not in sys.path:
    sys.path.insert(0, "/opt/trn_rl_repo")


def _force_combined_act_table():
    """Point walrus at an act_info.json holding only natural_log_exp_and_others
    (contains exp+ln), so every ACTIVATE shares one table."""
    if os.environ.get("BASS_ACT_ROOT_JSON_PATH"):
        return
    from neuronxcc.driver.Job import Job  # type: ignore

    pwp = None
    for cand in glob.glob(os.path.join(Job.getPackageDir(), "pwp", "pwp_bin_*")):
        if os.path.exists(os.path.join(cand, "act_info.json")):
            pwp = cand
            break
    if pwp is None:
        return
    info = json.load(open(os.path.join(pwp, "act_info.json")))
    keep = [t for t in info.get("act_func_sets", [])
            if t.get("name") == "natural_log_exp_and_others"]
    if not keep:
        return
    out_dir = os.path.join(tempfile.gettempdir(), "dli_act_combined")
    os.makedirs(out_dir, exist_ok=True)
    for t in keep:
        for k in info.get("pwp_file_keys", []):
            f = t.get(k)
            src = os.path.join(pwp, f) if f else None
            if src and os.path.exists(src):
                dst = os.path.join(out_dir, f)
                if not os.path.exists(dst):
                    shutil.copy(src, dst)
    info = dict(info)
    info["act_func_sets"] = keep
    with open(os.path.join(out_dir, "act_info.json"), "w") as f:
        json.dump(info, f)
    os.environ["BASS_ACT_ROOT_JSON_PATH"] = os.path.join(out_dir, "act_info.json")


_force_combined_act_table()

from contextlib import ExitStack

import numpy as np

import concourse.bacc as bacc
import concourse.bass as bass
import concourse.hw_specs as hw_specs
import concourse.mybir as mybir
import concourse.tile as tile

# Make bass's act-table placement agree with the trimmed act_info.json walrus
# sees: only the combined exp+ln table exists, so every ACTIVATE maps to
# act_func_set_id 0 and the table is loaded exactly once.
_orig_get_act_tables = hw_specs.get_activation_tables


def _combined_act_tables(module_arch):
    tabs = _orig_get_act_tables(module_arch)
    kept = {k: v for k, v in tabs.items() if k == "natural_log_exp_and_others"}
    return kept if kept and os.environ.get("BASS_ACT_ROOT_JSON_PATH") else tabs


hw_specs.get_activation_tables = _combined_act_tables
bacc.get_activation_tables = _combined_act_tables

# Cheaper kernel teardown: drain + one all-engine barrier + sem clear. The
# stock epilogue adds a second all-engine barrier after the clear; engines
# that pass the first barrier only run their terminal branch, and the next
# execution starts only after every engine (incl. the clearing one) halts,
# so the second barrier only adds ~4us of EVSEM latency.
from concourse.vector_clock import ScopedClock as _ScopedClock


def _cheap_drain_and_barrier(self, tick_clock, wait_clock):
    drain_inst = self.nc.sync.drain()
    wait_clock.add_sem_waits(
        drain_inst.ins, _ScopedClock({None: tick_clock.global_clock})
    )
    self.nc.all_engine_barrier()
    popped = self.nc._tile_sem_poison_stack.pop()
    assert popped is self._sem_poison
    self.nc.clear_and_free_semaphores(list(self.sems.allocated().values()))


tile.TileContext._drain_and_barrier = _cheap_drain_and_barrier

F32 = mybir.dt.float32
F32R = mybir.dt.float32r
FP16 = mybir.dt.float16
ALU = mybir.AluOpType
ACTF = mybir.ActivationFunctionType
AXX = mybir.AxisListType.X

BSZ, L, D = 64, 128, 2048
N_CORES = 8
NB = BSZ // N_CORES  # batches per core
NCH = D // 128  # 16 contraction chunks
NEG = -30000.0  # additive suppression; exp(NEG + anything sane) == 0 in f32
ABID = 512.0  # batch-id suppression scale; ABID*bid is fp16-exact for bid<=7


def build_program(R):
    """R = packed+padded row count per core (multiple of 128, >= 256)."""
    T = R // 128  # row tiles
    nc = bacc.Bacc("TRN2", target_bir_lowering=False, debug=False, num_devices=1)

    enc = nc.dram_tensor("enc", [D, R], F32R, kind="ExternalInput").ap()
    wuv = nc.dram_tensor("wuv", [128, 2 * NCH], F32R, kind="ExternalInput").ap()
    # aux: rows = negated diagonal masks m1n/m2n (read as a 2-partition block)
    aux = nc.dram_tensor("aux", [2, R], F32, kind="ExternalInput").ap()
    # auxc: u-side bias addend columns, auxc[p, a] = c - ABID*bid[128a+p] | NEG
    auxc = nc.dram_tensor("auxc", [128, T], F32, kind="ExternalInput").ap()
    # bp: k-side rank-1 row, +ABID*bid | NEG (fp16-exact values)
    bp = nc.dram_tensor("bp", [1, R], F32, kind="ExternalInput").ap()
    out = nc.dram_tensor("out", [1, 1], F32, kind="ExternalOutput").ap()

    # psum column slices of <=512 f32 (one 2KB bank each)
    slices = [(s, min(s + 512, R)) for s in range(0, R, 512)]
    n_sl = len(slices)
    sl_of_tile = [min(128 * a // 512, n_sl - 1) for a in range(T)]

    with tile.TileContext(nc) as tc, ExitStack() as ctx:
        consts = ctx.enter_context(tc.tile_pool(name="consts", bufs=1))
        accs = ctx.enter_context(tc.tile_pool(name="accs", bufs=1))
        enc_pool = ctx.enter_context(tc.tile_pool(name="enc", bufs=1))
        junk_pool = ctx.enter_context(tc.tile_pool(name="junk", bufs=2))
        psum_uv_pool = ctx.enter_context(tc.tile_pool(name="psuv", bufs=1, space="PSUM"))
        psum_d_pool = ctx.enter_context(
            tc.tile_pool(name="psd", bufs=max(2, 4 - n_sl), space="PSUM"))
        psum_misc = ctx.enter_context(tc.tile_pool(name="psm", bufs=1, space="PSUM"))

        # ---- input DMAs (all HWDGE on the sync queue, in order) ----
        wuv_sb = consts.tile([128, 2 * NCH], F32R)
        nc.sync.dma_start(wuv_sb[:], wuv[:])
        aux_sb = consts.tile([2, R], F32)
        nc.sync.dma_start(aux_sb[:], aux[:])
        auxc_sb = consts.tile([128, T], F32)
        nc.sync.dma_start(auxc_sb[:], auxc[:])
        bp_sb = consts.tile([1, R], F32)
        nc.sync.dma_start(bp_sb[:], bp[:])
        enc_tiles = []
        for ch in range(NCH):
            t = enc_pool.tile([128, R], F32R, name=f"encc{ch}", tag=f"enc{ch}")
            nc.sync.dma_start(t[:], enc[128 * ch : 128 * (ch + 1), :])
            enc_tiles.append(t)

        # ---- constants (GpSimd; overlaps the DMAs) ----
        ones16 = consts.tile([1, 128], FP16)
        nc.gpsimd.memset(ones16[:], 1.0)
        ident16 = consts.tile([128, 128], FP16)
        nc.gpsimd.memset(ident16[:], 1.0)
        nc.gpsimd.affine_select(
            out=ident16[:], in_=ident16[:], compare_op=ALU.is_equal, fill=0.0,
            base=0, pattern=[[-1, 128]], channel_multiplier=1,
        )
        triNEG16 = consts.tile([128, 128], FP16)
        nc.gpsimd.memset(triNEG16[:], 0.0)
        nc.gpsimd.affine_select(
            out=triNEG16[:], in_=triNEG16[:], compare_op=ALU.is_gt, fill=NEG,
            base=0, pattern=[[-1, 128]], channel_multiplier=1,
        )
        ones_col = consts.tile([128, 1], F32)
        nc.gpsimd.memset(ones_col[:], 1.0)
        ident2 = consts.tile([2, 2], F32)
        nc.gpsimd.memset(ident2[:], 1.0)
        nc.gpsimd.affine_select(
            out=ident2[:], in_=ident2[:], compare_op=ALU.is_equal, fill=0.0,
            base=0, pattern=[[-1, 2]], channel_multiplier=1,
        )
        ident128 = consts.tile([128, 128], F32)
        nc.gpsimd.memset(ident128[:], 1.0)
        nc.gpsimd.affine_select(
            out=ident128[:], in_=ident128[:], compare_op=ALU.is_equal, fill=0.0,
            base=0, pattern=[[-1, 128]], channel_multiplier=1,
        )

        # bp16[k] = +ABID*bid[k] (+c folded host-side into the u bias instead),
        # NEG on padding; fp16-exact values by construction.
        bp16 = accs.tile([1, R], FP16)
        nc.vector.tensor_copy(bp16[:], bp_sb[:])

        # ---- u,v dots on the PE: psum_uv[2, R] over 16 d-chunks ----
        psum_uv = []
        for s, (c0, c1) in enumerate(slices):
            pt = psum_uv_pool.tile([2, c1 - c0], F32, tag=f"uv{s}", name=f"uv{s}")
            psum_uv.append(pt)
            for ch in range(NCH):
                nc.tensor.matmul(
                    pt[:], lhsT=wuv_sb[:, 2 * ch : 2 * ch + 2],
                    rhs=enc_tiles[ch][:, c0:c1],
                    start=(ch == 0), stop=(ch == NCH - 1),
                )

        # ---- extract u (bias columns, f32) and v (fp16 rows) from psum_uv.
        # Engine APs must start at partition 0/32/64/96, so row 1 of psum_uv
        # cannot be read directly; go through small PE transposes instead:
        # [2,128] -> [128,2] per tile (u,v as columns), then v column ->
        # [1,128] row via an identity transpose (baseline pattern). ----
        uv_sb = accs.tile([2, R], F32)
        for s, (c0, c1) in enumerate(slices):
            nc.vector.tensor_copy(uv_sb[:, c0:c1], psum_uv[s][:])
        psum_t = psum_misc.tile([128, 2 * T], F32, tag="uc", name="uc")
        for a in range(T):
            nc.tensor.matmul(
                psum_t[:, 2 * a : 2 * a + 2],
                lhsT=uv_sb[0:2, 128 * a : 128 * (a + 1)],
                rhs=ident2[:], is_transpose=True,
            )
        # u bias columns: u + c - ABID*bid (NEG on pads)
        ubias = accs.tile([128, T], F32)
        nc.vector.tensor_add(ubias[:], psum_t[:, 0 : 2 * T : 2], auxc_sb[:])
        vcols = accs.tile([128, T], F32)
        nc.vector.tensor_copy(vcols[:], psum_t[:, 1 : 2 * T : 2])
        psum_vr_pool = ctx.enter_context(
            tc.tile_pool(name="psvr", bufs=2, space="PSUM"))
        v16 = accs.tile([1, R], FP16)
        for a in range(T):
            pvr = psum_vr_pool.tile([1, 128], F32, tag="vr")
            nc.tensor.matmul(
                pvr[:], lhsT=vcols[:, a : a + 1],
                rhs=ident128[:], is_transpose=True,
            )
            nc.vector.tensor_copy(v16[:, 128 * a : 128 * (a + 1)], pvr[:])

        # ---- phase B: pair blocks ----
        blocks = []  # (j_tile, k_tile)
        for a in range(T):
            blocks.append((a, a))
            if a + 1 < T:
                blocks.append((a + 1, a))
        # Ln groups: batch the row-sum reductions; keep the last group short.
        if len(blocks) >= 5:
            groups = [blocks[:3], blocks[3:-1], blocks[-1:]]
        else:
            groups = [blocks[:-1], blocks[-1:]] if len(blocks) > 1 else [blocks]
        groups = [g for g in groups if g]

        RS = accs.tile([128, len(groups)], F32)
        exg_tiles = [
            accs.tile([128, 128 * len(g)], F32, tag=f"exg{i}", name=f"exg{i}")
            for i, g in enumerate(groups)
        ]

        for gi, grp in enumerate(groups):
            for q, (a, b) in enumerate(grp):
                pd = psum_d_pool.tile([128, 128], F32, tag="pd")
                vs = v16[:, 128 * b : 128 * (b + 1)]
                bs = bp16[:, 128 * b : 128 * (b + 1)]
                if a == b:
                    nc.tensor.matmul(pd[:], lhsT=ident16[:], rhs=triNEG16[:],
                                     start=True, stop=False)
                    nc.tensor.matmul(pd[:], lhsT=ones16[:], rhs=vs,
                                     start=False, stop=False)
                    nc.tensor.matmul(pd[:], lhsT=ones16[:], rhs=bs,
                                     start=False, stop=True)
                else:
                    nc.tensor.matmul(pd[:], lhsT=ones16[:], rhs=vs,
                                     start=True, stop=False)
                    nc.tensor.matmul(pd[:], lhsT=ones16[:], rhs=bs,
                                     start=False, stop=True)
                nc.scalar.activation(
                    exg_tiles[gi][:, 128 * q : 128 * (q + 1)], pd[:], ACTF.Exp,
                    bias=ubias[:, a : a + 1],
                )
            sp = junk_pool.tile([128, 128 * len(grp)], F32, tag="sp")
            nc.scalar.activation(sp[:], exg_tiles[gi][:], ACTF.Ln, bias=1.0,
                                 accum_out=RS[:, gi : gi + 1])

        # ---- label-1 diagonal: -(sum u*m1 + sum v*m2) via negated masks ----
        uvm = accs.tile([2, R], F32)
        nc.vector.tensor_mul(uvm[:], uv_sb[:], aux_sb[:])
        dsum = accs.tile([2, 1], F32)
        nc.vector.reduce_sum(dsum[:], uvm[:], axis=AXX)

        # ---- final reduction ----
        accA = accs.tile([128, 1], F32)
        nc.vector.reduce_sum(accA[:], RS[:], axis=AXX)
        psum_s = psum_misc.tile([1, 1], F32, tag="psm", name="psm")
        nc.tensor.matmul(psum_s[:], lhsT=accA[:], rhs=ones_col[:],
                         start=True, stop=False)
        nc.tensor.matmul(psum_s[:], lhsT=dsum[:], rhs=ones_col[0:2, :],
                         start=False, stop=True)
        out_t = accs.tile([1, 1], F32)
        nc.vector.tensor_copy(out_t[:], psum_s[:])
        nc.sync.dma_start(out[:], out_t[:])

    nc.compile()
    return nc


_NC_CACHE = {}
_LAST_R = None


def _get_nc(R=None):
    global _LAST_R
    if R is None:
        R = _LAST_R if _LAST_R is not None else 512
    if R not in _NC_CACHE:
        _NC_CACHE[R] = build_program(R)
    _LAST_R = R
    return _NC_CACHE[R]


def _prep(encoder_output, mask, W, b):
    """Host-side prep: length-balanced shard + packed transposed layout."""
    enc = np.asarray(encoder_output, dtype=np.float32)
    W = np.asarray(W, dtype=np.float32)
    b = np.asarray(b, dtype=np.float32).reshape(2)
    mask = np.asarray(mask)
    c = float(b[1] - b[0])
    lens = mask.astype(np.int64).sum(axis=1)  # [BSZ]

    # greedy length-balance into N_CORES groups of NB
    order = np.argsort(-lens, kind="stable")
    assign = [[] for _ in range(N_CORES)]
    loads = [0] * N_CORES
    for bidx in order:
        for cid in sorted(range(N_CORES), key=lambda q: (loads[q], q)):
            if len(assign[cid]) < NB:
                assign[cid].append(int(bidx))
                loads[cid] += int(lens[bidx])
                break
    R = max(256, -(-max(loads) // 128) * 128)

    wd = np.stack([W[:D, 1] - W[:D, 0], W[D:, 1] - W[D:, 0]])  # [2, D]
    wuvT = np.empty((128, 2 * NCH), dtype=np.float32)
    for ch in range(NCH):
        wuvT[:, 2 * ch] = wd[0, 128 * ch : 128 * (ch + 1)]
        wuvT[:, 2 * ch + 1] = wd[1, 128 * ch : 128 * (ch + 1)]

    T = R // 128
    maps = []
    for cid in range(N_CORES):
        buf = np.zeros((R, D), dtype=np.float32)
        auxm = np.zeros((2, R), dtype=np.float32)  # m1n, m2n (negated masks)
        addu = np.full(R, NEG, dtype=np.float32)   # u-side bias addend
        bpr = np.full(R, NEG, dtype=np.float32)    # k-side rank-1 row
        off = 0
        for beta, bidx in enumerate(assign[cid]):
            ln_ = int(lens[bidx])
            buf[off : off + ln_] = enc[bidx, :ln_]
            addu[off : off + ln_] = c - ABID * beta
            bpr[off : off + ln_] = ABID * beta
            auxm[0, off + 1 : off + ln_] = -1.0    # m1 negated (not first row)
            auxm[1, off : off + ln_ - 1] = -1.0    # m2 negated (not last row)
            off += ln_
        maps.append(
            {
                "enc": np.ascontiguousarray(buf.T),
                "wuv": wuvT,
                "aux": auxm,
                "auxc": np.ascontiguousarray(addu.reshape(T, 128).T),
                "bp": bpr.reshape(1, R),
            }
        )
    diag_c = float(c * (lens - 1).sum())
    n_valid = int((lens * (lens - 1) // 2).sum())
    return maps, diag_c, n_valid, R


def kernel(encoder_output, mask, W, b, _run_kwargs=None):
    from concourse.bass_utils import run_bass_kernel_spmd

    maps, diag_c, n_valid, R = _prep(np.asarray(encoder_output), mask, W, b)
    nc = _get_nc(R)
    res = run_bass_kernel_spmd(nc, maps, core_ids=list(range(N_CORES)),
                               **(_run_kwargs or {}))
    total = float(sum(np.float64(r["out"][0, 0]) for r in res.results))
    total -= diag_c
    loss = total / max(n_valid, 1)
    out = np.array(loss, dtype=np.float32)
    if _run_kwargs is not None:
        return out, res
    return out


# revision 14
# speedup vs baseline: 1.2834x; 1.1711x over previous
"""Trainium2 Bass kernel for the DLI (dialogue-turn ordering) loss — v2.

Math (exact reduction of the reference):
  With 2 classes, NLL(label y) = softplus(l_{1-y} - l_y).
  u[b,j] = enc[b,j] @ (W[:D,1]-W[:D,0]),
  v[b,k] = enc[b,k] @ (W[D:,1]-W[D:,0]),
  c      = b[1]-b[0],  d[b,j,k] = u[b,j] + v[b,k] + c
  label = 1 iff k == j-1; valid pairs: k < j < len_b;  softplus(-d) = softplus(d) - d
  =>  sum_nll = sum_{valid} softplus(d) - sum_{b, 1<=j<len_b} d[b,j,j-1]
  loss = sum_nll / max(n_valid, 1)

v2 layout (ragged packing + PE dots):
  Only rows j < len_b ever matter (arch is ragged_sequence).  The host
  length-balances the 64 batches into 8 cores of 8, packs each core's valid
  rows contiguously, and ships them TRANSPOSED: encT [D=2048, R] f32 where
  R = packed rows padded to a multiple of 128 (512 for the expected data).
  HBM traffic is halved vs the full [8, 128, 2048] and the transposed layout
  lets the TensorEngine do the dots:

    psum_uv[2, R] += wuvT_chunk[128, 2].T @ encT_chunk[128, R]   (16 chunks)

  as f32r matmuls (1 cycle/row at R >= 256), replacing the DVE/ACT dot
  pipeline of v1 that was the bottleneck (ACT 30us + DVE 30us busy).  All
  DMAs are plain-dtype HWDGE on the sync queue (no SWDGE => no GpSimd
  descriptor work, no dge drains).

  Phase B on packed coordinates: tiles of 128 rows; pair blocks (a,a) and
  (a,a-1) only (a batch spans at most 2 tiles).  d is built additively in
  PSUM by rank-1 fp16 matmuls:
    diag:  ident@triNEG + ones@v16 + ones@bp16      (tri kills k>=j)
    off :  ones@v16 + ones@bp16                     (all global k < j)
  where bp16[k] = +A*bid[k] (A=512, fp16-exact) or NEG for padding, and the
  Exp bias column carries u[j] + c - A*bid[j] in f32 (via a K=1 PE
  transpose of the u row).  Cross-batch pairs get exp(d - A*dbid) -> 0;
  same-batch pairs cancel the A terms exactly.  Exp/Ln(1+x) on ACT with
  fused row-sum accumulation; label-1 diagonal handled as masked row sums
  of raw u,v (masks shipped negated so the final PSUM matmul accumulates
  the subtraction).  Host divides by exact n_valid and fixes the c terms.
"""

import glob
import json
import os
import shutil
import sys
import tempfile

if "/opt/trn_rl_repo" not in sys.path:
    sys.path.insert(0, "/opt/trn_rl_repo")


def _force_combined_act_table():
    """Point walrus at an act_info.json holding only natural_log_exp_and_others
    (contains exp+ln), so every ACTIVATE shares one table."""
    if os.environ.get("BASS_ACT_ROOT_JSON_PATH"):
        return
    from neuronxcc.driver.Job import Job  # type: ignore

    pwp = None
    for cand in glob.glob(os.path.join(Job.getPackageDir(), "pwp", "pwp_bin_*")):
        if os.path.exists(os.path.join(cand, "act_info.json")):
            pwp = cand
            break
    if pwp is None:
        return
    info = json.load(open(os.path.join(pwp, "act_info.json")))
    keep = [t for t in info.get("act_func_sets", [])
            if t.get("name") == "natural_log_exp_and_others"]
    if not keep:
        return
    out_dir = os.path.join(tempfile.gettempdir(), "dli_act_combined")
    os.makedirs(out_dir, exist_ok=True)
    for t in keep:
        for k in info.get("pwp_file_keys", []):
            f = t.get(k)
            src = os.path.join(pwp, f) if f else None
            if src and os.path.exists(src):
                dst = os.path.join(out_dir, f)
                if not os.path.exists(dst):
                    shutil.copy(src, dst)
    info = dict(info)
    info["act_func_sets"] = keep
    with open(os.path.join(out_dir, "act_info.json"), "w") as f:
        json.dump(info, f)
    os.environ["BASS_ACT_ROOT_JSON_PATH"] = os.path.join(out_dir, "act_info.json")


_force_combined_act_table()

from contextlib import ExitStack

import numpy as np

import concourse.bacc as bacc
import concourse.bass as bass
import concourse.hw_specs as hw_specs
import concourse.mybir as mybir
import concourse.tile as tile

# Make bass's act-table placement agree with the trimmed act_info.json walrus
# sees: only the combined exp+ln table exists, so every ACTIVATE maps to
# act_func_set_id 0 and the table is loaded exactly once.
_orig_get_act_tables = hw_specs.get_activation_tables


def _combined_act_tables(module_arch):
    tabs = _orig_get_act_tables(module_arch)
    kept = {k: v for k, v in tabs.items() if k == "natural_log_exp_and_others"}
    return kept if kept and os.environ.get("BASS_ACT_ROOT_JSON_PATH") else tabs


hw_specs.get_activation_tables = _combined_act_tables
bacc.get_activation_tables = _combined_act_tables

# Cheaper kernel teardown: drain + one all-engine barrier + sem clear. The
# stock epilogue adds a second all-engine barrier after the clear; engines
# that pass the first barrier only run their terminal branch, and the next
# execution starts only after every engine (incl. the clearing one) halts,
# so the second barrier only adds ~4us of EVSEM latency.
from concourse.vector_clock import ScopedClock as _ScopedClock


def _cheap_drain_and_barrier(self, tick_clock, wait_clock):
    drain_inst = self.nc.sync.drain()
    wait_clock.add_sem_waits(
        drain_inst.ins, _ScopedClock({None: tick_clock.global_clock})
    )
    self.nc.all_engine_barrier()
    popped = self.nc._tile_sem_poison_stack.pop()
    assert popped is self._sem_poison
    self.nc.clear_and_free_semaphores(list(self.sems.allocated().values()))


tile.TileContext._drain_and_barrier = _cheap_drain_and_barrier

F32 = mybir.dt.float32
BF16 = mybir.dt.bfloat16
FP16 = mybir.dt.float16
ALU = mybir.AluOpType
ACTF = mybir.ActivationFunctionType
AXX = mybir.AxisListType.X

BSZ, L, D = 64, 128, 2048
N_CORES = 8
NB = BSZ // N_CORES  # batches per core
NCH = D // 128  # 16 contraction chunks
NEG = -30000.0  # additive suppression; exp(NEG + anything sane) == 0 in f32
ABID = 512.0  # batch-id suppression scale; ABID*bid is fp16-exact for bid<=7


def build_program(R):
    """R = packed+padded row count per core (multiple of 128, >= 256)."""
    T = R // 128  # row tiles
    W32 = 2 * NCH + 3 * T + 3 + 128  # wuv | auxc | mcols | ones_col | ident2 | ident128
    nc = bacc.Bacc("TRN2", target_bir_lowering=False, debug=False, num_devices=1)

    enc = nc.dram_tensor("enc", [D, R], F32, kind="ExternalInput").ap()
    # host-packed constants; see _prep for column layouts
    sf32 = nc.dram_tensor("sf32", [128, W32], F32, kind="ExternalInput").ap()
    sf16c = nc.dram_tensor("sf16c", [128, 256], FP16, kind="ExternalInput").ap()
    sf16r = nc.dram_tensor("sf16r", [1, R + 128], FP16, kind="ExternalInput").ap()
    out = nc.dram_tensor("out", [1, 1], F32, kind="ExternalOutput").ap()

    # psum column slices of <=512 f32 (one 2KB bank each)
    slices = [(s, min(s + 512, R)) for s in range(0, R, 512)]
    n_sl = len(slices)

    NG = 4  # enc DMA groups (DMA_DIRECT2D triggers cost ~750ns serially)
    CPG = NCH // NG  # chunks per group

    with tile.TileContext(nc) as tc, ExitStack() as ctx:
        consts = ctx.enter_context(tc.tile_pool(name="consts", bufs=1))
        accs = ctx.enter_context(tc.tile_pool(name="accs", bufs=1))
        enc_pool = ctx.enter_context(tc.tile_pool(name="enc", bufs=1))
        junk_pool = ctx.enter_context(tc.tile_pool(name="junk", bufs=2))
        psum_uv_pool = ctx.enter_context(tc.tile_pool(name="psuv", bufs=1, space="PSUM"))
        psum_d_pool = ctx.enter_context(
            tc.tile_pool(name="psd", bufs=max(2, 4 - n_sl), space="PSUM"))
        psum_vr_pool = ctx.enter_context(tc.tile_pool(name="psvr", bufs=2, space="PSUM"))
        psum_misc = ctx.enter_context(tc.tile_pool(name="psm", bufs=1, space="PSUM"))

        # ---- small-constant DMAs on the scalar-engine HWDGE queue ----
        sf32_sb = consts.tile([128, W32], F32)
        nc.scalar.dma_start(sf32_sb[:], sf32[:])
        sf16c_sb = consts.tile([128, 256], FP16)
        nc.scalar.dma_start(sf16c_sb[:], sf16c[:])
        sf16r_sb = consts.tile([1, R + 128], FP16)
        nc.scalar.dma_start(sf16r_sb[:], sf16r[:])
        auxc_sb = sf32_sb[:, 2 * NCH : 2 * NCH + T]           # u bias addend cols
        mcols = sf32_sb[:, 2 * NCH + T : 2 * NCH + 3 * T]     # negated diag masks
        ones_col = sf32_sb[:, 2 * NCH + 3 * T : 2 * NCH + 3 * T + 1]
        ident2 = sf32_sb[0:2, 2 * NCH + 3 * T + 1 : 2 * NCH + 3 * T + 3]
        ident128 = sf32_sb[:, W32 - 128 : W32]
        ident16 = sf16c_sb[:, 0:128]
        triNEG16 = sf16c_sb[:, 128:256]
        bp16 = sf16r_sb[0:1, 0:R]
        ones16 = sf16r_sb[0:1, R : R + 128]

        # ---- enc: NG grouped DMAs (sync queue), then per-chunk bf16 casts ----
        encf, encb = [], []
        for g in range(NG):
            tf = enc_pool.tile([128, CPG * R], F32, name=f"encf{g}", tag=f"encf{g}")
            nc.sync.dma_start(
                tf[:].rearrange("p (c r) -> p c r", c=CPG),
                enc[128 * CPG * g : 128 * CPG * (g + 1), :].rearrange(
                    "(c p) r -> p c r", p=128),
            )
            tb = enc_pool.tile([128, CPG * R], BF16, name=f"encb{g}", tag=f"encb{g}")
            encf.append(tf)
            encb.append(tb)
        wuv_bf = consts.tile([128, 2 * NCH], BF16)
        nc.vector.tensor_copy(wuv_bf[:], sf32_sb[:, 0 : 2 * NCH])
        for ch in range(NCH):
            g, cl = divmod(ch, CPG)
            nc.vector.tensor_copy(
                encb[g][:, cl * R : (cl + 1) * R], encf[g][:, cl * R : (cl + 1) * R])

        # ---- u,v dots on the PE: psum_uv[2, R] over the 16 d-chunks ----
        psum_uv = []
        for s, (c0, c1) in enumerate(slices):
            pt = psum_uv_pool.tile([2, c1 - c0], F32, tag=f"uv{s}", name=f"uv{s}")
            psum_uv.append(pt)
            for ch in range(NCH):
                g, cl = divmod(ch, CPG)
                nc.tensor.matmul(
                    pt[:], lhsT=wuv_bf[:, 2 * ch : 2 * ch + 2],
                    rhs=encb[g][:, cl * R + c0 : cl * R + c1],
                    start=(ch == 0), stop=(ch == NCH - 1),
                )

        # ---- extract u (bias columns, f32) and v (fp16 rows) from psum_uv.
        # Engine APs must start at partition 0/32/64/96, so row 1 of psum_uv
        # cannot be read directly; go through small PE transposes instead. ----
        uv_sb = accs.tile([2, R], F32)
        for s, (c0, c1) in enumerate(slices):
            nc.vector.tensor_copy(uv_sb[:, c0:c1], psum_uv[s][:])
        psum_t = psum_misc.tile([128, 2 * T], F32, tag="uc", name="uc")
        for a in range(T):
            nc.tensor.matmul(
                psum_t[:, 2 * a : 2 * a + 2],
                lhsT=uv_sb[0:2, 128 * a : 128 * (a + 1)],
                rhs=ident2, is_transpose=True,
            )
        # u bias columns: u + c - ABID*bid (NEG on pads)
        ubias = accs.tile([128, T], F32)
        nc.vector.tensor_add(ubias[:], psum_t[:, 0 : 2 * T : 2], auxc_sb)
        vcols = accs.tile([128, T], F32)
        nc.vector.tensor_copy(vcols[:], psum_t[:, 1 : 2 * T : 2])
        v16 = accs.tile([1, R], FP16)
        for a in range(T):
            pvr = psum_vr_pool.tile([1, 128], F32, tag="vr")
            nc.tensor.matmul(pvr[:], lhsT=vcols[:, a : a + 1],
                             rhs=ident128, is_transpose=True)
            nc.vector.tensor_copy(v16[:, 128 * a : 128 * (a + 1)], pvr[:])

        # ---- phase B: pair blocks ----
        blocks = []  # (j_tile, k_tile)
        for a in range(T):
            blocks.append((a, a))
            if a + 1 < T:
                blocks.append((a + 1, a))
        if len(blocks) >= 5:
            groups = [blocks[:3], blocks[3:-1], blocks[-1:]]
        else:
            groups = [blocks[:-1], blocks[-1:]] if len(blocks) > 1 else [blocks]
        groups = [g for g in groups if g]

        RS = accs.tile([128, len(groups)], F32)
        exg_tiles = [
            accs.tile([128, 128 * len(g)], F32, tag=f"exg{i}", name=f"exg{i}")
            for i, g in enumerate(groups)
        ]

        for gi, grp in enumerate(groups):
            for q, (a, b) in enumerate(grp):
                pd = psum_d_pool.tile([128, 128], F32, tag="pd")
                vs = v16[:, 128 * b : 128 * (b + 1)]
                bs = bp16[:, 128 * b : 128 * (b + 1)]
                if a == b:
                    nc.tensor.matmul(pd[:], lhsT=ident16, rhs=triNEG16,
                                     start=True, stop=False)
                    nc.tensor.matmul(pd[:], lhsT=ones16, rhs=vs,
                                     start=False, stop=False)
                    nc.tensor.matmul(pd[:], lhsT=ones16, rhs=bs,
                                     start=False, stop=True)
                else:
                    nc.tensor.matmul(pd[:], lhsT=ones16, rhs=vs,
                                     start=True, stop=False)
                    nc.tensor.matmul(pd[:], lhsT=ones16, rhs=bs,
                                     start=False, stop=True)
                nc.scalar.activation(
                    exg_tiles[gi][:, 128 * q : 128 * (q + 1)], pd[:], ACTF.Exp,
                    bias=ubias[:, a : a + 1],
                )
            sp = junk_pool.tile([128, 128 * len(grp)], F32, tag="sp")
            nc.scalar.activation(sp[:], exg_tiles[gi][:], ACTF.Ln, bias=1.0,
                                 accum_out=RS[:, gi : gi + 1])

        # ---- label-1 diagonal in column space: -(sum u*m1 + sum v*m2) ----
        uvm = accs.tile([128, 2 * T], F32)
        nc.vector.tensor_mul(uvm[:], psum_t[:], mcols)
        dsumc = accs.tile([128, 1], F32)
        nc.vector.reduce_sum(dsumc[:], uvm[:], axis=AXX)

        # ---- final reduction ----
        accA = accs.tile([128, 1], F32)
        nc.vector.reduce_sum(accA[:], RS[:], axis=AXX)
        nc.vector.tensor_add(accA[:], accA[:], dsumc[:])
        psum_s = psum_misc.tile([1, 1], F32, tag="psm", name="psm")
        nc.tensor.matmul(psum_s[:], lhsT=accA[:], rhs=ones_col,
                         start=True, stop=True)
        out_t = accs.tile([1, 1], F32)
        nc.vector.tensor_copy(out_t[:], psum_s[:])
        nc.sync.dma_start(out[:], out_t[:])

    nc.compile()
    return nc


_NC_CACHE = {}
_LAST_R = None


def _get_nc(R=None):
    global _LAST_R
    if R is None:
        R = _LAST_R if _LAST_R is not None else 512
    if R not in _NC_CACHE:
        _NC_CACHE[R] = build_program(R)
    _LAST_R = R
    return _NC_CACHE[R]


def _prep(encoder_output, mask, W, b):
    """Host-side prep: length-balanced shard + packed transposed layout."""
    enc = np.asarray(encoder_output, dtype=np.float32)
    W = np.asarray(W, dtype=np.float32)
    b = np.asarray(b, dtype=np.float32).reshape(2)
    mask = np.asarray(mask)
    c = float(b[1] - b[0])
    lens = mask.astype(np.int64).sum(axis=1)  # [BSZ]

    # greedy length-balance into N_CORES groups of NB
    order = np.argsort(-lens, kind="stable")
    assign = [[] for _ in range(N_CORES)]
    loads = [0] * N_CORES
    for bidx in order:
        for cid in sorted(range(N_CORES), key=lambda q: (loads[q], q)):
            if len(assign[cid]) < NB:
                assign[cid].append(int(bidx))
                loads[cid] += int(lens[bidx])
                break
    R = max(256, -(-max(loads) // 128) * 128)
    T = R // 128
    W32 = 2 * NCH + 3 * T + 3 + 128

    wd = np.stack([W[:D, 1] - W[:D, 0], W[D:, 1] - W[D:, 0]])  # [2, D]

    # fp16 constants: ident | triNEG columns, and (bp | ones) row per core
    j = np.arange(128)
    sf16c = np.zeros((128, 256), dtype=np.float16)
    sf16c[j, j] = 1.0
    sf16c[:, 128:256] = np.where(j[None, :] < j[:, None], 0.0, NEG).astype(np.float16)

    maps = []
    for cid in range(N_CORES):
        buf = np.zeros((R, D), dtype=np.float32)
        addu = np.full(R, NEG, dtype=np.float32)   # u-side bias addend
        bpr = np.full(R, NEG, dtype=np.float32)    # k-side rank-1 row
        m1n = np.zeros(R, dtype=np.float32)
        m2n = np.zeros(R, dtype=np.float32)
        off = 0
        for beta, bidx in enumerate(assign[cid]):
            ln_ = int(lens[bidx])
            buf[off : off + ln_] = enc[bidx, :ln_]
            addu[off : off + ln_] = c - ABID * beta
            bpr[off : off + ln_] = ABID * beta
            m1n[off + 1 : off + ln_] = -1.0        # m1 negated (not first row)
            m2n[off : off + ln_ - 1] = -1.0        # m2 negated (not last row)
            off += ln_
        sf32 = np.zeros((128, W32), dtype=np.float32)
        sf32[:, 0 : 2 * NCH] = wd.T.reshape(NCH, 128, 2).transpose(
            1, 0, 2).reshape(128, 2 * NCH)
        sf32[:, 2 * NCH : 2 * NCH + T] = addu.reshape(T, 128).T
        mc = np.stack([m1n.reshape(T, 128).T, m2n.reshape(T, 128).T], axis=2)
        sf32[:, 2 * NCH + T : 2 * NCH + 3 * T] = mc.reshape(128, 2 * T)
        sf32[:, 2 * NCH + 3 * T] = 1.0             # ones_col
        sf32[0, 2 * NCH + 3 * T + 1] = 1.0         # ident2
        sf32[1, 2 * NCH + 3 * T + 2] = 1.0
        sf32[np.arange(128), W32 - 128 + np.arange(128)] = 1.0  # ident128
        sf16r = np.zeros((1, R + 128), dtype=np.float16)
        sf16r[0, :R] = bpr
        sf16r[0, R:] = 1.0
        maps.append(
            {
                "enc": np.ascontiguousarray(buf.T),
                "sf32": sf32,
                "sf16c": sf16c,
                "sf16r": sf16r,
            }
        )
    diag_c = float(c * (lens - 1).sum())
    n_valid = int((lens * (lens - 1) // 2).sum())
    return maps, diag_c, n_valid, R


def kernel(encoder_output, mask, W, b, _run_kwargs=None):
    from concourse.bass_utils import run_bass_kernel_spmd

    maps, diag_c, n_valid, R = _prep(np.asarray(encoder_output), mask, W, b)
    nc = _get_nc(R)
    res = run_bass_kernel_spmd(nc, maps, core_ids=list(range(N_CORES)),
                               **(_run_kwargs or {}))
    total = float(sum(np.float64(r["out"][0, 0]) for r in res.results))
    total -= diag_c
    loss = total / max(n_valid, 1)
    out = np.array(loss, dtype=np.float32)
    if _run_kwargs is not None:
        return out, res
    return out
